# revision 15
# baseline (speedup 1.0000x reference)
"""v6 Trainium2 Bass kernel for nn_Decoder_75093208203382.

out[e] = relu((RNA[ridx[e]] * PROT[pidx[e]]) @ W_comb),  W_comb [128, 4]

Lineage of measurements that drove the design:
  - v3 (q0 SBUF transposed per-edge gather): 585 us — the wall is 62.5K
    SDMA descriptors/core at ~9 ns/desc on ONE SWDGE queue.
  - SWDGE has 4 queues; queues 1-3 produce CORRUPT data in transposed
    gather mode (desc-gen misreads the idx array; verified per-queue:
    q0 always right, q1+ always wrong, both SBUF- and HBM-source).
  - NON-transposed HBM-source gather is CORRECT on all 4 queues and
    runs at ~1.8-2.5 ns/desc aggregate (scratch-size dependent).

v6 therefore:
  - RNA side: non-transposed HBM gather of bf16 rows on queues ci%4 ->
    r_nt[e%128, e//128, d], then ONE hardware XBAR transpose DMA per
    chunk (14 ns per 16x128 tile) -> r_td[d, b, e] for the DVE multiply.
  - Protein side unchanged from v3: host sorts edges by protein index,
    1024-edge groups fit a 128-row window; PE expands rows to per-edge
    columns with a bf16 one-hot matmul (zero descriptors).
  - Projection per 128-block on PE, relu on ACT, one output DMA/chunk.
    Host unsorts the result.
"""

import contextlib
import os
import sys

import numpy as np

for _p in ("/opt/trn_rl_repo", "/root/.axon_site/_ro/trn_rl_repo"):
    if os.path.isdir(_p) and _p not in sys.path:
        sys.path.insert(0, _p)
        break

import concourse.bacc as bacc
import concourse.mybir as mybir
import concourse.tile as tile
from concourse.bass_utils import run_bass_kernel_spmd

N_CORES = 8
E_TOTAL = 500000
E_CORE = E_TOTAL // N_CORES       # 62500
D = 128
C = 4
GRP = 1024                        # edges per protein window group
CHUNKS = [2048] * 30 + [1152]     # per-gather-call edges; sum = 62592
E_PAD = sum(CHUNKS)
N_GRP = (E_PAD + GRP - 1) // GRP  # 62 (61 full + one 128-edge tail)
N_RNA = 20000
N_PROT = 5000
DMA_SCRATCH = 98304
N_QUEUES = 4

F32 = mybir.dt.float32
BF16 = mybir.dt.bfloat16
I16 = mybir.dt.int16
F8 = mybir.dt.float8e4


def _emit_body(nc, chunks, rna_rows, pwin_sb, ident, s_d, ridx_sb, wc, out,
               gpool, rtpool, spool, rspool, wpool, ppool, xpool, opool):
    col = 0    # index-tile column offset
    oc = 0     # output column offset
    eb = 0     # global edge offset (for group ids)
    # Pending projection matmuls (thunks). Each proj's 128-col weight load
    # only hides if the PREVIOUS PE matmul streams >=128 moving columns, so
    # projs are interleaved into the next group's transpose+expansion MMs.
    pending = []
    for ci, n in enumerate(chunks):
        nblk = n // 128
        r_nt = gpool.tile([128, nblk, 2 * D], BF16, tag="rg")
        nc.gpsimd.dma_gather(
            r_nt[:], rna_rows[:], ridx_sb[:, col : col + n // 16],
            num_idxs=n, num_idxs_reg=n, elem_size=2 * D,
            transpose=False, single_packet=False, queue_num=ci % N_QUEUES,
        )
        s_t = spool.tile([128, n], F8, tag="s")
        nc.scalar.dma_start(s_t[:], s_d[:, eb : eb + n])
        r_sb = rspool.tile([128, n], BF16, tag="rsb")
        rp = wpool.tile([128, 1, n], BF16, tag="rp")
        ps = ppool.tile([128, nblk, 4], F32, tag="ps")

        for a in range(0, n, GRP):
            w = min(GRP, n - a)
            nb = w // 128
            g = (eb + a) // GRP
            pp = xpool.tile([128, 1, GRP], F32, tag="pp")
            rt = rtpool.tile([128, nb, 128], BF16, tag="rt")
            for b8 in range(nb):
                lo = 128 * b8
                # transpose r_nt block: [e,d] -> [d,e] (PSUM, bf16)
                nc.tensor.transpose(
                    rt[:, b8, :], r_nt[:, (a + lo) // 128, 0:D], ident[:],
                )
                # protein one-hot expansion block
                nc.tensor.matmul(
                    pp[:, 0, lo : lo + 128], lhsT=pwin_sb[:, g, :],
                    rhs=s_t[:, a + lo : a + lo + 128],
                    start=True, stop=True,
                )
                if pending:
                    pending.pop(0)()
            # PSUM -> SBUF so DVE reads one PSUM operand max
            nc.scalar.copy(r_sb[:, a : a + w], rt[:])
            nc.vector.tensor_tensor(
                out=rp[:, :, a : a + w],
                in0=r_sb[:, a : a + w],
                in1=pp[:, :, 0:w],
                op=mybir.AluOpType.mult,
            )

            def mk_proj(ps_t, a_, lo_, rp_t):
                def emit():
                    nc.tensor.matmul(
                        ps_t[:, (a_ + lo_) // 128, :],
                        lhsT=rp_t[:, 0, a_ + lo_ : a_ + lo_ + 128],
                        rhs=wc[:], start=True, stop=True,
                    )
                return emit

            for b8 in range(nb):
                pending.append(mk_proj(ps, a, 128 * b8, rp))

        # flush enough pendings that ps(ci) completes before relu(ci):
        # everything still pending belongs to this chunk's last group.
        while pending:
            pending.pop(0)()
        stage = opool.tile([128, nblk, 4], F32, tag="stage")
        nc.scalar.activation(
            stage[:], ps[:], mybir.ActivationFunctionType.Relu,
        )
        nc.sync.dma_start(out[:, oc : oc + 4 * nblk], stage[:])
        col += n // 16
        oc += 4 * nblk
        eb += n


def build_kernel(chunks=None, reps=1, scratch=DMA_SCRATCH):
    chunks = list(chunks) if chunks is not None else list(CHUNKS)
    e_pad = sum(chunks)
    n_grp = (e_pad + GRP - 1) // GRP
    nc = bacc.Bacc("TRN2", target_bir_lowering=False, debug=False,
                   dynamic_dma_scratch_size=scratch,
                   num_swdge_queues=N_QUEUES)

    # each row stored twice (512B): 512B gather descriptors measured ~25%
    # faster per-desc than 256B; downstream reads the first half only.
    rna_rows = nc.dram_tensor("rna_rows", [N_RNA, 2 * D], BF16,
                              kind="ExternalInput")
    ident_d = nc.dram_tensor("ident", [128, 128], BF16, kind="ExternalInput")
    pwin = nc.dram_tensor("pwin", [128, n_grp, 128], BF16, kind="ExternalInput")
    s_d = nc.dram_tensor("s_d", [128, e_pad], F8, kind="ExternalInput")
    rna_idx = nc.dram_tensor("rna_idx", [128, e_pad // 16], I16, kind="ExternalInput")
    wcomb = nc.dram_tensor("wcomb", [D, C], BF16, kind="ExternalInput")
    n_oc = (e_pad // 128) * 4
    out = nc.dram_tensor("out", [128, n_oc], F32, kind="ExternalOutput")

    with tile.TileContext(nc) as tc:
        with (
            tc.tile_pool(name="const", bufs=1) as cpool,
            tc.tile_pool(name="gather", bufs=5) as gpool,
            tc.tile_pool(name="rtp", bufs=2, space="PSUM") as rtpool,
            tc.tile_pool(name="sel", bufs=3) as spool,
            tc.tile_pool(name="rsb", bufs=2) as rspool,
            tc.tile_pool(name="work", bufs=2) as wpool,
            tc.tile_pool(name="psum", bufs=2, space="PSUM") as ppool,
            tc.tile_pool(name="xpsum", bufs=2, space="PSUM") as xpool,
            tc.tile_pool(name="outp", bufs=2) as opool,
        ):
            pwin_sb = cpool.tile([128, n_grp, 128], BF16, tag="pwin_sb")
            nc.sync.dma_start(pwin_sb[:], pwin[:])
            ident = cpool.tile([128, 128], BF16, tag="ident")
            nc.sync.dma_start(ident[:], ident_d[:])
            wc = cpool.tile([D, C], BF16, tag="wc")
            nc.sync.dma_start(wc[:], wcomb[:])
            ridx_sb = cpool.tile([128, e_pad // 16], I16, tag="ridx")
            nc.sync.dma_start(ridx_sb[:], rna_idx[:])

            loop_cm = tc.For_i(0, reps, 1) if reps > 1 else contextlib.nullcontext()
            with loop_cm:
                _emit_body(nc, chunks, rna_rows, pwin_sb, ident, s_d,
                           ridx_sb, wc, out, gpool, rtpool, spool, rspool,
                           wpool, ppool, xpool, opool)

    nc.compile()
    return nc


def _wrap_indices(idx, chunks):
    cols = []
    base = 0
    for n in chunks:
        cols.append(idx[base : base + n].reshape(n // 16, 16).T)
        base += n
    wrapped = np.hstack(cols).astype(np.int16)
    return np.tile(wrapped, (8, 1))


def _f32_to_bf16(a):
    import ml_dtypes

    return a.astype(ml_dtypes.bfloat16)


_NC_CACHE = {}


def _get_nc():
    key = "default"
    if key not in _NC_CACHE:
        _NC_CACHE[key] = build_kernel()
    return _NC_CACHE[key]


def prep_in_maps(RNA_inputs, protein_inputs, RNA_indices, protein_indices,
                 w_relation, weight_classifier):
    import ml_dtypes

    _rr = _f32_to_bf16(np.ascontiguousarray(
        np.asarray(RNA_inputs, dtype=np.float32)))      # [N_RNA, D]
    rna_rows = np.ascontiguousarray(np.concatenate([_rr, _rr], axis=1))
    prot_bf = _f32_to_bf16(np.asarray(protein_inputs, dtype=np.float32))
    ridx = np.asarray(RNA_indices).astype(np.int64)
    pidx = np.asarray(protein_indices).astype(np.int64)
    wrel = np.asarray(w_relation, dtype=np.float64)
    wcls = np.asarray(weight_classifier, dtype=np.float64)
    wcomb = _f32_to_bf16(np.ascontiguousarray((wrel.T @ wcls).astype(np.float32)))

    in_maps = []
    orders = []
    for c in range(N_CORES):
        lo = c * E_CORE
        rc = ridx[lo : lo + E_CORE]
        pc = pidx[lo : lo + E_CORE]
        order = np.argsort(pc, kind="stable")
        orders.append(order)
        rs = np.zeros(E_PAD, dtype=np.int64)
        pss = np.zeros(E_PAD, dtype=np.int64)
        rs[:E_CORE] = rc[order]
        pss[:E_CORE] = pc[order]
        pss[E_CORE:] = pss[E_CORE - 1]  # pads extend the last window
        # group windows
        bases = np.empty(N_GRP, dtype=np.int64)
        s_rows = np.empty(E_PAD, dtype=np.int64)
        for g in range(N_GRP):
            a, b = g * GRP, min((g + 1) * GRP, E_PAD)
            mn, mx = pss[a:b].min(), pss[a:b].max()
            assert mx - mn < 128, (c, g, mn, mx)
            base = max(0, mx - 127)
            bases[g] = base
            s_rows[a:b] = pss[a:b] - base
        # pwin[u, g, d] = prot[base_g + u, d]
        pw = prot_bf[np.minimum(bases[None, :] + np.arange(128)[:, None],
                                N_PROT - 1)]  # [128, N_GRP, D]
        # S one-hot [128, E_PAD]
        s_mat = np.zeros((128, E_PAD), dtype=ml_dtypes.float8_e4m3)
        s_mat[s_rows, np.arange(E_PAD)] = 1.0
        in_maps.append({
            "rna_rows": rna_rows,
            "ident": _f32_to_bf16(np.eye(128, dtype=np.float32)),
            "pwin": np.ascontiguousarray(pw),
            "s_d": s_mat,
            "rna_idx": _wrap_indices(rs, CHUNKS),
            "wcomb": wcomb,
        })
    prep_in_maps.orders = orders
    return in_maps


def run_nc(nc, in_maps):
    return run_bass_kernel_spmd(nc, in_maps, core_ids=list(range(N_CORES)))


def postprocess(res, orders):
    parts = []
    for c, r in enumerate(res.results):
        arr = np.asarray(r["out"], dtype=np.float32)  # [128, 4*E_PAD/128]
        nblk = arr.shape[1] // 4
        edges = arr.reshape(128, nblk, 4).transpose(1, 0, 2).reshape(-1, 4)
        unsorted = np.empty((E_CORE, 4), dtype=np.float32)
        unsorted[orders[c]] = edges[:E_CORE]
        parts.append(unsorted)
    return np.ascontiguousarray(np.concatenate(parts, axis=0).astype(np.float32))


def kernel(RNA_inputs, protein_inputs, RNA_indices, protein_indices,
           w_relation, weight_classifier):
    in_maps = prep_in_maps(RNA_inputs, protein_inputs, RNA_indices,
                           protein_indices, w_relation, weight_classifier)
    res = run_nc(_get_nc(), in_maps)
    return postprocess(res, prep_in_maps.orders)
